# revision 10
# baseline (speedup 1.0000x reference)
"""MoE (top-2 of 8 experts) Trainium2 kernel.

Strategy: expert-parallel across the 8 NeuronCores. The router (a tiny
[T,512]@[512,8] matmul + softmax + top-k, ~0.02% of the layer's FLOPs) runs
on host bit-identically to the reference (jax on CPU). Tokens are gathered
per expert on host, padded to a common capacity C, and each core computes
its expert's full FFN on device:

    outT = (w2.T @ gelu(w1.T @ xT + b1) + b2) * gate

in a transposed layout (features on partitions, tokens on the moving/free
axis) so both matmuls chain on the TensorEngine with no transposes, and the
b1/b2 biases are free per-partition operands. The gate multiply uses a
partition-broadcast gate row. Host scatter-adds the two expert
contributions per token back into the full [B,S,D] output.

Only the selected top-2 experts contribute to the reference output (the
gate is exactly zero elsewhere), so this computes 4x fewer FLOPs than the
dense reference while being numerically equivalent.
"""

import os
import sys

sys.path.insert(0, "/opt/trn_rl_repo")

import numpy as np

TOP_K = 2
N_CORES = 8
P = 128  # SBUF partitions

# Matmul dtype: "float32" (exact, 4 cyc/row) or "float32r" (1 cyc/row at
# N>=256, reduced internal precision). Overridable for experiments.
MM_DT = os.environ.get("MOE_MM_DT", "float32")
NTILE = 512  # moving-operand (token) tile; max for 4-byte dtypes
# n-tiles processed per weight pass (fp32/fp32r matmuls self-load weights,
# so >1 only helps dtypes with separate LDWEIGHTS)
NPAIR = int(os.environ.get("MOE_NPAIR", "1"))


def _route(x_flat, gate_w, gate_b):
    """Reference router, bit-identical: jax on CPU."""
    import jax
    import jax.numpy as jnp

    with jax.default_device(jax.devices("cpu")[0]):
        logits = jnp.asarray(x_flat) @ jnp.asarray(gate_w) + jnp.asarray(gate_b)
        raw_weights = jax.nn.softmax(logits, axis=-1)
        top_w, top_idx = jax.lax.top_k(raw_weights, TOP_K)
        return np.asarray(top_w), np.asarray(top_idx)


ACT_FUNC = os.environ.get("MOE_ACT_FUNC", "Gelu")  # CoreSim lacks Gelu; Tanh for sim


def _build_program(C, D, H, mm_dt_name):
    """Build the per-core Bass program (identical on all cores)."""
    import concourse.bass as bass
    import concourse.mybir as mybir
    import concourse.tile as tile
    from concourse import bacc

    f32 = mybir.dt.float32
    mm_dt = getattr(mybir.dt, mm_dt_name)
    KT = D // P  # 4  k-tiles for matmul1 (contraction over D)
    MT = H // P  # 16 m-tiles (H rows of hT)
    DT = D // P  # 4  d-tiles of the output
    NT = (C + NTILE - 1) // NTILE

    nc = bacc.Bacc(None, target_bir_lowering=False, debug=False)
    xt_h = nc.dram_tensor("xt", [D, C], mm_dt, kind="ExternalInput")
    g_h = nc.dram_tensor("g", [1, C], f32, kind="ExternalInput")
    w1_h = nc.dram_tensor("w1", [D, H], mm_dt, kind="ExternalInput")
    b1_h = nc.dram_tensor("b1", [P, MT], f32, kind="ExternalInput")
    w2_h = nc.dram_tensor("w2", [H, D], mm_dt, kind="ExternalInput")
    b2_h = nc.dram_tensor("b2", [P, DT], f32, kind="ExternalInput")
    out_h = nc.dram_tensor("out", [D, C], f32, kind="ExternalOutput")

    w1_r = w1_h.ap().rearrange("(kt p) h -> p kt h", p=P)  # [128, KT, H]
    w2_r = w2_h.ap().rearrange("(mt p) d -> p mt d", p=P)  # [128, MT, D]
    xt_r = xt_h.ap().rearrange("(kt p) c -> p kt c", p=P)  # [128, KT, C]
    out_r = out_h.ap().rearrange("(dt p) c -> p dt c", p=P)  # [128, DT, C]

    with tile.TileContext(nc) as tc:
        with (
            tc.tile_pool(name="weights", bufs=1) as wpool,
            tc.tile_pool(name="xio", bufs=2) as xio,
            tc.tile_pool(name="gio", bufs=2) as gio,
            tc.tile_pool(name="oio", bufs=3) as oio,
            tc.tile_pool(name="hbuf", bufs=1) as hbuf,
            tc.tile_pool(name="ps1", bufs=2, space=bass.MemorySpace.PSUM) as ps1,
            tc.tile_pool(name="ps2", bufs=2, space=bass.MemorySpace.PSUM) as ps2,
        ):
            w1_sb = wpool.tile([P, KT, H], mm_dt)
            for kt in range(KT):
                nc.sync.dma_start(out=w1_sb[:, kt, :], in_=w1_r[:, kt, :])
            w2_sb = wpool.tile([P, MT, D], mm_dt)
            for mt in range(MT):
                nc.sync.dma_start(out=w2_sb[:, mt, :], in_=w2_r[:, mt, :])
            b1_sb = wpool.tile([P, MT], f32)
            nc.sync.dma_start(out=b1_sb, in_=b1_h.ap())
            b2_sb = wpool.tile([P, DT], f32)
            nc.sync.dma_start(out=b2_sb, in_=b2_h.ap())

            for n0 in range(0, NT, NPAIR):
                npair = min(NPAIR, NT - n0)
                # token slice covered by this group of n-tiles
                c0 = n0 * NTILE
                csz = min(NPAIR * NTILE, C - c0)
                xt_t = xio.tile([P, KT, csz], mm_dt, tag="xt")
                nc.sync.dma_start(out=xt_t, in_=xt_r[:, :, c0 : c0 + csz])
                g_t = gio.tile([P, csz], f32, tag="g")
                nc.gpsimd.dma_start(
                    out=g_t, in_=g_h.ap()[:, c0 : c0 + csz].partition_broadcast(P)
                )
                hT = hbuf.tile([P, MT, csz], mm_dt, tag="hT")
                nsz = [
                    min(NTILE, csz - i * NTILE)
                    for i in range((csz + NTILE - 1) // NTILE)
                ]
                for m in range(MT):
                    pst = [
                        ps1.tile([P, s], f32, tag=f"ps1_{i}", name=f"ps1_{i}")
                        for i, s in enumerate(nsz)
                    ]
                    for kt in range(KT):
                        lhs = w1_sb[:, kt, P * m : P * (m + 1)]
                        for i, s in enumerate(nsz):
                            nc.tensor.matmul(
                                pst[i],
                                lhsT=lhs,
                                rhs=xt_t[:, kt, i * NTILE : i * NTILE + s],
                                start=(kt == 0),
                                stop=(kt == KT - 1),
                            )
                    for i, s in enumerate(nsz):
                        nc.scalar.activation(
                            out=hT[:, m, i * NTILE : i * NTILE + s],
                            in_=pst[i],
                            func=getattr(mybir.ActivationFunctionType, ACT_FUNC),
                            bias=b1_sb[:, m : m + 1],
                            scale=1.0,
                        )
                for d in range(DT):
                    pso = [
                        ps2.tile([P, s], f32, tag=f"ps2_{i}", name=f"ps2_{i}")
                        for i, s in enumerate(nsz)
                    ]
                    for m in range(MT):
                        lhs = w2_sb[:, m, P * d : P * (d + 1)]
                        for i, s in enumerate(nsz):
                            nc.tensor.matmul(
                                pso[i],
                                lhsT=lhs,
                                rhs=hT[:, m, i * NTILE : i * NTILE + s],
                                start=(m == 0),
                                stop=(m == MT - 1),
                            )
                    ot = oio.tile([P, csz], f32, tag="ot")
                    for i, s in enumerate(nsz):
                        nc.vector.scalar_tensor_tensor(
                            out=ot[:, i * NTILE : i * NTILE + s],
                            in0=pso[i],
                            scalar=b2_sb[:, d : d + 1],
                            in1=g_t[:, i * NTILE : i * NTILE + s],
                            op0=mybir.AluOpType.add,
                            op1=mybir.AluOpType.mult,
                        )
                    nc.sync.dma_start(out=out_r[:, d, c0 : c0 + csz], in_=ot)

    nc.compile()
    return nc


def _run(nc, in_maps, trace=False):
    from concourse.bass_utils import run_bass_kernel_spmd

    if trace:
        # register the NTFF profiling hook (missing antenv.axon_hooks shim)
        import types

        import antenv

        if not hasattr(antenv, "axon_hooks"):
            mod = types.ModuleType("antenv.axon_hooks")
            _hook = [None]
            mod.set_axon_ntff_profile_hook = lambda h: _hook.__setitem__(0, h)
            mod.get_axon_ntff_profile_hook = lambda: _hook[0]
            sys.modules["antenv.axon_hooks"] = mod
            antenv.axon_hooks = mod
            from trn_agent_boot.trn_boot import _ntff_profile_via_ctypes

            mod.set_axon_ntff_profile_hook(
                _ntff_profile_via_ctypes("/opt/axon/libaxon_pjrt.so")
            )
    return run_bass_kernel_spmd(
        nc, in_maps, core_ids=list(range(N_CORES)), trace=trace
    )


def kernel(x, gate_w, gate_b, w1, b1, w2, b2, _trace=False):
    x = np.ascontiguousarray(np.asarray(x, dtype=np.float32))
    gate_w = np.asarray(gate_w, dtype=np.float32)
    gate_b = np.asarray(gate_b, dtype=np.float32)
    w1 = np.asarray(w1, dtype=np.float32)
    b1 = np.asarray(b1, dtype=np.float32)
    w2 = np.asarray(w2, dtype=np.float32)
    b2 = np.asarray(b2, dtype=np.float32)

    B, S, D = x.shape
    E = gate_w.shape[1]
    H = w1.shape[2]
    assert E == N_CORES
    T = B * S
    x_flat = x.reshape(T, D)

    top_w, top_idx = _route(x_flat, gate_w, gate_b)

    toks, gvals = [], []
    for e in range(E):
        mask = top_idx == e  # [T, K]; at most one True per row
        t_ids = np.nonzero(mask.any(axis=1))[0]
        toks.append(t_ids)
        gvals.append(top_w[mask].astype(np.float32))
    Cmax = max(len(t) for t in toks)
    C = max(((Cmax + P - 1) // P) * P, NTILE)

    in_maps = []
    for e in range(E):
        cnt = len(toks[e])
        XT = np.zeros((D, C), np.float32)
        XT[:, :cnt] = x_flat[toks[e]].T
        G = np.zeros((1, C), np.float32)
        G[0, :cnt] = gvals[e]
        MT, DT = H // P, D // P
        in_maps.append(
            {
                "xt": XT,
                "g": G,
                "w1": np.ascontiguousarray(w1[e]),
                "b1": np.ascontiguousarray(b1[e].reshape(MT, P).T),
                "w2": np.ascontiguousarray(w2[e]),
                "b2": np.ascontiguousarray(b2[e].reshape(DT, P).T),
            }
        )

    nc = _build_program(C, D, H, MM_DT)
    res = _run(nc, in_maps, trace=_trace)
    global _LAST_RES
    _LAST_RES = res

    out_flat = np.zeros((T, D), np.float32)
    for e in range(E):
        cnt = len(toks[e])
        out_flat[toks[e]] += res.results[e]["out"][:, :cnt].T

    out = out_flat.reshape(B, S, D)
    if _trace:
        return out, res.exec_time_ns
    return out


# revision 11
# speedup vs baseline: 1.0597x; 1.0597x over previous
"""MoE (top-2 of 8 experts) Trainium2 kernel.

Strategy: expert-parallel across the 8 NeuronCores. The router (a tiny
[T,512]@[512,8] matmul + softmax + top-k, ~0.02% of the layer's FLOPs) runs
on host bit-identically to the reference (jax on CPU). Tokens are gathered
per expert on host, padded to a common capacity C, and each core computes
its expert's full FFN on device:

    outT = (w2.T @ gelu(w1.T @ xT + b1) + b2) * gate

in a transposed layout (features on partitions, tokens on the moving/free
axis) so both matmuls chain on the TensorEngine with no transposes, and the
b1/b2 biases are free per-partition operands. The gate multiply uses a
partition-broadcast gate row. Host scatter-adds the two expert
contributions per token back into the full [B,S,D] output.

Only the selected top-2 experts contribute to the reference output (the
gate is exactly zero elsewhere), so this computes 4x fewer FLOPs than the
dense reference while being numerically equivalent.
"""

import os
import sys

sys.path.insert(0, "/opt/trn_rl_repo")

import numpy as np

TOP_K = 2
N_CORES = 8
P = 128  # SBUF partitions

# Matmul dtype: "float32" (exact, 4 cyc/row) or "float32r" (1 cyc/row at
# N>=256, reduced internal precision). Overridable for experiments.
MM_DT = os.environ.get("MOE_MM_DT", "float32")
NTILE = 512  # moving-operand (token) tile; max for 4-byte dtypes
# n-tiles processed per weight pass (fp32/fp32r matmuls self-load weights,
# so >1 only helps dtypes with separate LDWEIGHTS)
NPAIR = int(os.environ.get("MOE_NPAIR", "1"))


def _route(x_flat, gate_w, gate_b):
    """Reference router, bit-identical: jax on CPU."""
    import jax
    import jax.numpy as jnp

    with jax.default_device(jax.devices("cpu")[0]):
        logits = jnp.asarray(x_flat) @ jnp.asarray(gate_w) + jnp.asarray(gate_b)
        raw_weights = jax.nn.softmax(logits, axis=-1)
        top_w, top_idx = jax.lax.top_k(raw_weights, TOP_K)
        return np.asarray(top_w), np.asarray(top_idx)


ACT_FUNC = os.environ.get("MOE_ACT_FUNC", "Gelu")  # CoreSim lacks Gelu; Tanh for sim


def _build_program(C, D, H, mm_dt_name):
    """Build the per-core Bass program (identical on all cores)."""
    import concourse.bass as bass
    import concourse.mybir as mybir
    import concourse.tile as tile
    from concourse import bacc

    f32 = mybir.dt.float32
    mm_dt = getattr(mybir.dt, mm_dt_name)
    KT = D // P  # 4  k-tiles for matmul1 (contraction over D)
    MT = H // P  # 16 m-tiles (H rows of hT)
    DT = D // P  # 4  d-tiles of the output
    NT = (C + NTILE - 1) // NTILE

    nc = bacc.Bacc(None, target_bir_lowering=False, debug=False)
    xt_h = nc.dram_tensor("xt", [D, C], mm_dt, kind="ExternalInput")
    g_h = nc.dram_tensor("g", [1, C], f32, kind="ExternalInput")
    w1_h = nc.dram_tensor("w1", [D, H], mm_dt, kind="ExternalInput")
    b1_h = nc.dram_tensor("b1", [P, MT], f32, kind="ExternalInput")
    w2_h = nc.dram_tensor("w2", [H, D], mm_dt, kind="ExternalInput")
    b2_h = nc.dram_tensor("b2", [P, DT], f32, kind="ExternalInput")
    out_h = nc.dram_tensor("out", [D, C], f32, kind="ExternalOutput")

    w1_r = w1_h.ap().rearrange("(kt p) h -> p kt h", p=P)  # [128, KT, H]
    w2_r = w2_h.ap().rearrange("(mt p) d -> p mt d", p=P)  # [128, MT, D]
    xt_r = xt_h.ap().rearrange("(kt p) c -> p kt c", p=P)  # [128, KT, C]
    out_r = out_h.ap().rearrange("(dt p) c -> p dt c", p=P)  # [128, DT, C]

    with tile.TileContext(nc) as tc:
        with (
            tc.tile_pool(name="weights", bufs=1) as wpool,
            tc.tile_pool(name="xio", bufs=2) as xio,
            tc.tile_pool(name="gio", bufs=2) as gio,
            tc.tile_pool(name="oio", bufs=3) as oio,
            tc.tile_pool(name="hbuf", bufs=1) as hbuf,
            tc.tile_pool(name="ps1", bufs=2, space=bass.MemorySpace.PSUM) as ps1,
            tc.tile_pool(name="ps2", bufs=2, space=bass.MemorySpace.PSUM) as ps2,
        ):
            # Load order matters: the first n-tile's x slice and the first
            # m-group of w1 land first so the PE starts ~6us in instead of
            # waiting out the full 8.4MB weight load (~33us).
            w1_sb = wpool.tile([P, KT, H], mm_dt)
            w2_sb = wpool.tile([P, MT, D], mm_dt)
            MG = 512  # w1 column chunk (4 m-tiles)
            xt_first = xio.tile([P, KT, min(NPAIR * NTILE, C)], mm_dt, tag="xt")
            for kt in range(KT):
                nc.sync.dma_start(
                    out=xt_first[:, kt, :], in_=xt_r[:, kt, 0 : xt_first.shape[2]]
                )
            for mg in range(H // MG):
                for kt in range(KT):
                    nc.sync.dma_start(
                        out=w1_sb[:, kt, MG * mg : MG * (mg + 1)],
                        in_=w1_r[:, kt, MG * mg : MG * (mg + 1)],
                    )
            b1_sb = wpool.tile([P, MT], f32)
            nc.sync.dma_start(out=b1_sb, in_=b1_h.ap())
            b2_sb = wpool.tile([P, DT], f32)
            nc.sync.dma_start(out=b2_sb, in_=b2_h.ap())
            for mt in range(MT):
                nc.sync.dma_start(out=w2_sb[:, mt, :], in_=w2_r[:, mt, :])

            for n0 in range(0, NT, NPAIR):
                npair = min(NPAIR, NT - n0)
                # token slice covered by this group of n-tiles
                c0 = n0 * NTILE
                csz = min(NPAIR * NTILE, C - c0)
                if n0 == 0:
                    xt_t = xt_first
                else:
                    xt_t = xio.tile([P, KT, csz], mm_dt, tag="xt")
                    for kt in range(KT):
                        nc.sync.dma_start(
                            out=xt_t[:, kt, :], in_=xt_r[:, kt, c0 : c0 + csz]
                        )
                g_t = gio.tile([P, csz], f32, tag="g")
                nc.gpsimd.dma_start(
                    out=g_t, in_=g_h.ap()[:, c0 : c0 + csz].partition_broadcast(P)
                )
                hT = hbuf.tile([P, MT, csz], mm_dt, tag="hT")
                nsz = [
                    min(NTILE, csz - i * NTILE)
                    for i in range((csz + NTILE - 1) // NTILE)
                ]
                for m in range(MT):
                    pst = [
                        ps1.tile([P, s], f32, tag=f"ps1_{i}", name=f"ps1_{i}")
                        for i, s in enumerate(nsz)
                    ]
                    for kt in range(KT):
                        lhs = w1_sb[:, kt, P * m : P * (m + 1)]
                        for i, s in enumerate(nsz):
                            nc.tensor.matmul(
                                pst[i],
                                lhsT=lhs,
                                rhs=xt_t[:, kt, i * NTILE : i * NTILE + s],
                                start=(kt == 0),
                                stop=(kt == KT - 1),
                            )
                    for i, s in enumerate(nsz):
                        nc.scalar.activation(
                            out=hT[:, m, i * NTILE : i * NTILE + s],
                            in_=pst[i],
                            func=getattr(mybir.ActivationFunctionType, ACT_FUNC),
                            bias=b1_sb[:, m : m + 1],
                            scale=1.0,
                        )
                for d in range(DT):
                    pso = [
                        ps2.tile([P, s], f32, tag=f"ps2_{i}", name=f"ps2_{i}")
                        for i, s in enumerate(nsz)
                    ]
                    for m in range(MT):
                        lhs = w2_sb[:, m, P * d : P * (d + 1)]
                        for i, s in enumerate(nsz):
                            nc.tensor.matmul(
                                pso[i],
                                lhsT=lhs,
                                rhs=hT[:, m, i * NTILE : i * NTILE + s],
                                start=(m == 0),
                                stop=(m == MT - 1),
                            )
                    ot = oio.tile([P, csz], f32, tag="ot")
                    for i, s in enumerate(nsz):
                        nc.vector.scalar_tensor_tensor(
                            out=ot[:, i * NTILE : i * NTILE + s],
                            in0=pso[i],
                            scalar=b2_sb[:, d : d + 1],
                            in1=g_t[:, i * NTILE : i * NTILE + s],
                            op0=mybir.AluOpType.add,
                            op1=mybir.AluOpType.mult,
                        )
                    nc.sync.dma_start(out=out_r[:, d, c0 : c0 + csz], in_=ot)

    nc.compile()
    return nc


def _run(nc, in_maps, trace=False):
    from concourse.bass_utils import run_bass_kernel_spmd

    if trace:
        # register the NTFF profiling hook (missing antenv.axon_hooks shim)
        import types

        import antenv

        if not hasattr(antenv, "axon_hooks"):
            mod = types.ModuleType("antenv.axon_hooks")
            _hook = [None]
            mod.set_axon_ntff_profile_hook = lambda h: _hook.__setitem__(0, h)
            mod.get_axon_ntff_profile_hook = lambda: _hook[0]
            sys.modules["antenv.axon_hooks"] = mod
            antenv.axon_hooks = mod
            from trn_agent_boot.trn_boot import _ntff_profile_via_ctypes

            mod.set_axon_ntff_profile_hook(
                _ntff_profile_via_ctypes("/opt/axon/libaxon_pjrt.so")
            )
    return run_bass_kernel_spmd(
        nc, in_maps, core_ids=list(range(N_CORES)), trace=trace
    )


def kernel(x, gate_w, gate_b, w1, b1, w2, b2, _trace=False):
    x = np.ascontiguousarray(np.asarray(x, dtype=np.float32))
    gate_w = np.asarray(gate_w, dtype=np.float32)
    gate_b = np.asarray(gate_b, dtype=np.float32)
    w1 = np.asarray(w1, dtype=np.float32)
    b1 = np.asarray(b1, dtype=np.float32)
    w2 = np.asarray(w2, dtype=np.float32)
    b2 = np.asarray(b2, dtype=np.float32)

    B, S, D = x.shape
    E = gate_w.shape[1]
    H = w1.shape[2]
    assert E == N_CORES
    T = B * S
    x_flat = x.reshape(T, D)

    top_w, top_idx = _route(x_flat, gate_w, gate_b)

    toks, gvals = [], []
    for e in range(E):
        mask = top_idx == e  # [T, K]; at most one True per row
        t_ids = np.nonzero(mask.any(axis=1))[0]
        toks.append(t_ids)
        gvals.append(top_w[mask].astype(np.float32))
    Cmax = max(len(t) for t in toks)
    C = max(((Cmax + P - 1) // P) * P, NTILE)

    in_maps = []
    for e in range(E):
        cnt = len(toks[e])
        XT = np.zeros((D, C), np.float32)
        XT[:, :cnt] = x_flat[toks[e]].T
        G = np.zeros((1, C), np.float32)
        G[0, :cnt] = gvals[e]
        MT, DT = H // P, D // P
        in_maps.append(
            {
                "xt": XT,
                "g": G,
                "w1": np.ascontiguousarray(w1[e]),
                "b1": np.ascontiguousarray(b1[e].reshape(MT, P).T),
                "w2": np.ascontiguousarray(w2[e]),
                "b2": np.ascontiguousarray(b2[e].reshape(DT, P).T),
            }
        )

    nc = _build_program(C, D, H, MM_DT)
    res = _run(nc, in_maps, trace=_trace)
    global _LAST_RES
    _LAST_RES = res

    out_flat = np.zeros((T, D), np.float32)
    for e in range(E):
        cnt = len(toks[e])
        out_flat[toks[e]] += res.results[e]["out"][:, :cnt].T

    out = out_flat.reshape(B, S, D)
    if _trace:
        return out, res.exec_time_ns
    return out


# revision 12
# speedup vs baseline: 1.0804x; 1.0196x over previous
"""MoE (top-2 of 8 experts) Trainium2 kernel.

Strategy: expert-parallel across the 8 NeuronCores. The router (a tiny
[T,512]@[512,8] matmul + softmax + top-k, ~0.02% of the layer's FLOPs) runs
on host bit-identically to the reference (jax on CPU). Tokens are gathered
per expert on host, padded to a common capacity C, and each core computes
its expert's full FFN on device:

    outT = (w2.T @ gelu(w1.T @ xT + b1) + b2) * gate

in a transposed layout (features on partitions, tokens on the moving/free
axis) so both matmuls chain on the TensorEngine with no transposes, and the
b1/b2 biases are free per-partition operands. The gate multiply uses a
partition-broadcast gate row. Host scatter-adds the two expert
contributions per token back into the full [B,S,D] output.

Only the selected top-2 experts contribute to the reference output (the
gate is exactly zero elsewhere), so this computes 4x fewer FLOPs than the
dense reference while being numerically equivalent.

All device inputs are packed on host into contiguous ~1MB blocks laid out
in exactly the order the kernel consumes them: HWDGE drains the sync ring
FIFO, so consumption-ordered contiguous blocks give both full DMA
bandwidth and earliest possible compute start.
"""

import os
import sys

sys.path.insert(0, "/opt/trn_rl_repo")

import numpy as np

TOP_K = 2
N_CORES = 8
P = 128  # SBUF partitions

# Matmul dtype: "float32" (exact, 4 cyc/row) or "float32r" (1 cyc/row at
# N>=256, TF32-like internal precision, ~2e-4 rel err end to end).
MM_DT = os.environ.get("MOE_MM_DT", "float32r")
NTILE = 512  # moving-operand (token) tile; max for 4-byte dtypes
MG = 512  # w1 column-block (4 m-tiles per block)
ACT_FUNC = os.environ.get("MOE_ACT_FUNC", "Gelu")  # CoreSim lacks Gelu; Tanh for sim


def _route(x_flat, gate_w, gate_b):
    """Reference router, bit-identical: jax on CPU."""
    import jax
    import jax.numpy as jnp

    with jax.default_device(jax.devices("cpu")[0]):
        logits = jnp.asarray(x_flat) @ jnp.asarray(gate_w) + jnp.asarray(gate_b)
        raw_weights = jax.nn.softmax(logits, axis=-1)
        top_w, top_idx = jax.lax.top_k(raw_weights, TOP_K)
        return np.asarray(top_w), np.asarray(top_idx)


def _tile_sizes(C):
    return [min(NTILE, C - c0) for c0 in range(0, C, NTILE)]


def _pack_inputs(XT, G, w1e, b1e, w2e, b2e, C, D, H):
    """Pack one expert's inputs into the kernel's blocked layouts."""
    KT, MT, DT = D // P, H // P, D // P
    MGn, MTG = H // MG, MT // 4
    xt_blocks = []
    for i, csz in enumerate(_tile_sizes(C)):
        c0 = i * NTILE
        xt_blocks.append(
            XT.reshape(KT, P, C)[:, :, c0 : c0 + csz].transpose(1, 0, 2).ravel()
        )
    return {
        "xt": np.ascontiguousarray(np.concatenate(xt_blocks)),
        "g": np.ascontiguousarray(G.reshape(1, C)),
        "w1": np.ascontiguousarray(
            w1e.reshape(KT, P, MGn, MG).transpose(2, 1, 0, 3)
        ),
        "b1": np.ascontiguousarray(b1e.reshape(MT, P).T),
        "w2": np.ascontiguousarray(
            w2e.reshape(MTG, 4, P, D).transpose(0, 2, 1, 3)
        ),
        "b2": np.ascontiguousarray(b2e.reshape(DT, P).T),
    }


def _unpack_out(flat, C, D):
    """Blocked per-(n,d) output -> outT [D, C]."""
    DT = D // P
    outT = np.empty((D, C), np.float32)
    off = 0
    for i, csz in enumerate(_tile_sizes(C)):
        c0 = i * NTILE
        for d in range(DT):
            outT[d * P : (d + 1) * P, c0 : c0 + csz] = flat[
                off : off + P * csz
            ].reshape(P, csz)
            off += P * csz
    return outT


def _build_program(C, D, H, mm_dt_name):
    """Build the per-core Bass program (identical on all cores)."""
    import concourse.bass as bass
    import concourse.mybir as mybir
    import concourse.tile as tile
    from concourse import bacc

    f32 = mybir.dt.float32
    mm_dt = getattr(mybir.dt, mm_dt_name)
    act = getattr(mybir.ActivationFunctionType, ACT_FUNC)
    KT = D // P  # 4  k-tiles for matmul1 (contraction over D)
    MT = H // P  # 16 m-tiles (H rows of hT)
    DT = D // P  # 4  d-tiles of the output
    MGn = H // MG  # 4  w1 column blocks
    MTG = MT // 4  # 4  w2 row-block groups
    sizes = _tile_sizes(C)
    NT = len(sizes)

    nc = bacc.Bacc(None, target_bir_lowering=False, debug=False)
    xt_h = nc.dram_tensor("xt", [P * KT * C], mm_dt, kind="ExternalInput")
    g_h = nc.dram_tensor("g", [1, C], f32, kind="ExternalInput")
    w1_h = nc.dram_tensor("w1", [MGn, P, KT, MG], mm_dt, kind="ExternalInput")
    b1_h = nc.dram_tensor("b1", [P, MT], f32, kind="ExternalInput")
    w2_h = nc.dram_tensor("w2", [MTG, P, 4, D], mm_dt, kind="ExternalInput")
    b2_h = nc.dram_tensor("b2", [P, DT], f32, kind="ExternalInput")
    out_h = nc.dram_tensor("out", [P * DT * C], f32, kind="ExternalOutput")

    with tile.TileContext(nc) as tc:
        with (
            tc.tile_pool(name="weights", bufs=1) as wpool,
            tc.tile_pool(name="xio", bufs=2) as xio,
            tc.tile_pool(name="gio", bufs=2) as gio,
            tc.tile_pool(name="oio", bufs=3) as oio,
            tc.tile_pool(name="hbuf", bufs=1) as hbuf,
            tc.tile_pool(name="ps1", bufs=2, space=bass.MemorySpace.PSUM) as ps1,
            tc.tile_pool(name="ps2", bufs=2, space=bass.MemorySpace.PSUM) as ps2,
        ):
            # DMA issue order == consumption order (sync ring is FIFO):
            # xt[n0], w1 blocks, then w2 blocks / biases, then per-n IO.
            w1_sb = wpool.tile([P, MGn, KT, MG], mm_dt)
            w2_sb = wpool.tile([P, MTG, 4, D], mm_dt)
            xt_tiles = {}
            xt_tiles[0] = xio.tile([P, KT, sizes[0]], mm_dt, tag="xt", name="xt0")
            nc.sync.dma_start(
                out=xt_tiles[0],
                in_=xt_h.ap()[0 : P * KT * sizes[0]].rearrange(
                    "(p kt c) -> p kt c", p=P, kt=KT
                ),
            )
            for mg in range(MGn):
                nc.sync.dma_start(out=w1_sb[:, mg], in_=w1_h.ap()[mg])
            b1_sb = wpool.tile([P, MT], f32)
            nc.sync.dma_start(out=b1_sb, in_=b1_h.ap())
            b2_sb = wpool.tile([P, DT], f32)
            nc.sync.dma_start(out=b2_sb, in_=b2_h.ap())
            for mtg in range(MTG):
                nc.sync.dma_start(out=w2_sb[:, mtg], in_=w2_h.ap()[mtg])

            xt_off = P * KT * sizes[0]
            out_off = 0
            for n in range(NT):
                csz = sizes[n]
                c0 = n * NTILE
                if n in xt_tiles:
                    xt_t = xt_tiles.pop(n)
                else:
                    xt_t = xio.tile([P, KT, csz], mm_dt, tag="xt", name="xt")
                    nc.sync.dma_start(
                        out=xt_t,
                        in_=xt_h.ap()[xt_off : xt_off + P * KT * csz].rearrange(
                            "(p kt c) -> p kt c", p=P, kt=KT
                        ),
                    )
                    xt_off += P * KT * csz
                g_t = gio.tile([P, csz], f32, tag="g", name="g_t")
                nc.gpsimd.dma_start(
                    out=g_t, in_=g_h.ap()[:, c0 : c0 + csz].partition_broadcast(P)
                )
                hT = hbuf.tile([P, MT, csz], mm_dt, tag="hT", name="hT")
                for m in range(MT):
                    pst = ps1.tile([P, csz], f32, tag="ps1", name="ps1")
                    for kt in range(KT):
                        nc.tensor.matmul(
                            pst,
                            lhsT=w1_sb[:, m // 4, kt, (m % 4) * P : (m % 4 + 1) * P],
                            rhs=xt_t[:, kt, :],
                            start=(kt == 0),
                            stop=(kt == KT - 1),
                        )
                    nc.scalar.activation(
                        out=hT[:, m, :],
                        in_=pst,
                        func=act,
                        bias=b1_sb[:, m : m + 1],
                        scale=1.0,
                    )
                for d in range(DT):
                    pso = ps2.tile([P, csz], f32, tag="ps2", name="ps2")
                    for m in range(MT):
                        nc.tensor.matmul(
                            pso,
                            lhsT=w2_sb[:, m // 4, m % 4, d * P : (d + 1) * P],
                            rhs=hT[:, m, :],
                            start=(m == 0),
                            stop=(m == MT - 1),
                        )
                    ot = oio.tile([P, csz], f32, tag="ot", name="ot")
                    nc.vector.scalar_tensor_tensor(
                        out=ot,
                        in0=pso,
                        scalar=b2_sb[:, d : d + 1],
                        in1=g_t,
                        op0=mybir.AluOpType.add,
                        op1=mybir.AluOpType.mult,
                    )
                    nc.sync.dma_start(
                        out=out_h.ap()[out_off : out_off + P * csz].rearrange(
                            "(p c) -> p c", p=P
                        ),
                        in_=ot,
                    )
                    out_off += P * csz

    nc.compile()
    return nc


def _run(nc, in_maps, trace=False):
    from concourse.bass_utils import run_bass_kernel_spmd

    if trace:
        # register the NTFF profiling hook (missing antenv.axon_hooks shim)
        import types

        import antenv

        if not hasattr(antenv, "axon_hooks"):
            mod = types.ModuleType("antenv.axon_hooks")
            _hook = [None]
            mod.set_axon_ntff_profile_hook = lambda h: _hook.__setitem__(0, h)
            mod.get_axon_ntff_profile_hook = lambda: _hook[0]
            sys.modules["antenv.axon_hooks"] = mod
            antenv.axon_hooks = mod
            from trn_agent_boot.trn_boot import _ntff_profile_via_ctypes

            mod.set_axon_ntff_profile_hook(
                _ntff_profile_via_ctypes("/opt/axon/libaxon_pjrt.so")
            )
    return run_bass_kernel_spmd(
        nc, in_maps, core_ids=list(range(N_CORES)), trace=trace
    )


def kernel(x, gate_w, gate_b, w1, b1, w2, b2, _trace=False):
    x = np.ascontiguousarray(np.asarray(x, dtype=np.float32))
    gate_w = np.asarray(gate_w, dtype=np.float32)
    gate_b = np.asarray(gate_b, dtype=np.float32)
    w1 = np.asarray(w1, dtype=np.float32)
    b1 = np.asarray(b1, dtype=np.float32)
    w2 = np.asarray(w2, dtype=np.float32)
    b2 = np.asarray(b2, dtype=np.float32)

    B, S, D = x.shape
    E = gate_w.shape[1]
    H = w1.shape[2]
    assert E == N_CORES
    T = B * S
    x_flat = x.reshape(T, D)

    top_w, top_idx = _route(x_flat, gate_w, gate_b)

    toks, gvals = [], []
    for e in range(E):
        mask = top_idx == e  # [T, K]; at most one True per row
        t_ids = np.nonzero(mask.any(axis=1))[0]
        toks.append(t_ids)
        gvals.append(top_w[mask].astype(np.float32))
    Cmax = max(len(t) for t in toks)
    C = max(((Cmax + P - 1) // P) * P, NTILE)

    in_maps = []
    for e in range(E):
        cnt = len(toks[e])
        XT = np.zeros((D, C), np.float32)
        XT[:, :cnt] = x_flat[toks[e]].T
        G = np.zeros((1, C), np.float32)
        G[0, :cnt] = gvals[e]
        in_maps.append(_pack_inputs(XT, G, w1[e], b1[e], w2[e], b2[e], C, D, H))

    nc = _build_program(C, D, H, MM_DT)
    res = _run(nc, in_maps, trace=_trace)
    global _LAST_RES
    _LAST_RES = res

    out_flat = np.zeros((T, D), np.float32)
    for e in range(E):
        cnt = len(toks[e])
        outT = _unpack_out(res.results[e]["out"], C, D)
        out_flat[toks[e]] += outT[:, :cnt].T

    out = out_flat.reshape(B, S, D)
    if _trace:
        return out, res.exec_time_ns
    return out


# revision 15
# speedup vs baseline: 1.1063x; 1.0239x over previous
"""MoE (top-2 of 8 experts) Trainium2 kernel.

Strategy: expert-parallel across the 8 NeuronCores. The router (a tiny
[T,512]@[512,8] matmul + softmax + top-k, ~0.02% of the layer's FLOPs) runs
on host bit-identically to the reference (jax on CPU). Tokens are gathered
per expert on host, padded to a common capacity C, and each core computes
its expert's full FFN on device:

    outT = (w2.T @ gelu(w1.T @ xT + b1) + b2) * gate

in a transposed layout (features on partitions, tokens on the moving/free
axis) so both matmuls chain on the TensorEngine with no transposes, and the
b1/b2 biases are free per-partition operands. The gate multiply uses a
partition-broadcast gate row. Host scatter-adds the two expert
contributions per token back into the full [B,S,D] output.

Only the selected top-2 experts contribute to the reference output (the
gate is exactly zero elsewhere), so this computes 4x fewer FLOPs than the
dense reference while being numerically equivalent.

All device inputs are packed on host into contiguous ~1MB blocks laid out
in exactly the order the kernel consumes them: HWDGE drains the sync ring
FIFO, so consumption-ordered contiguous blocks give both full DMA
bandwidth and earliest possible compute start.
"""

import os
import sys

sys.path.insert(0, "/opt/trn_rl_repo")

import numpy as np

TOP_K = 2
N_CORES = 8
P = 128  # SBUF partitions

# Matmul dtype: "float32" (exact, 4 cyc/row) or "float32r" (1 cyc/row at
# N>=256, TF32-like internal precision, ~2e-4 rel err end to end).
MM_DT = os.environ.get("MOE_MM_DT", "float32r")
NTILE = 512  # moving-operand (token) tile; max for 4-byte dtypes
MG = 512  # w1 column-block (4 m-tiles per block)
ACT_FUNC = os.environ.get("MOE_ACT_FUNC", "Gelu")  # CoreSim lacks Gelu; Tanh for sim


def _route(x_flat, gate_w, gate_b):
    """Reference router, bit-identical: jax on CPU."""
    import jax
    import jax.numpy as jnp

    with jax.default_device(jax.devices("cpu")[0]):
        logits = jnp.asarray(x_flat) @ jnp.asarray(gate_w) + jnp.asarray(gate_b)
        raw_weights = jax.nn.softmax(logits, axis=-1)
        top_w, top_idx = jax.lax.top_k(raw_weights, TOP_K)
        return np.asarray(top_w), np.asarray(top_idx)


def _tile_sizes(C):
    return [min(NTILE, C - c0) for c0 in range(0, C, NTILE)]


def _pack_inputs(XT, G, w1e, b1e, w2e, b2e, C, D, H):
    """Pack one expert's inputs into the kernel's blocked layouts."""
    KT, MT, DT = D // P, H // P, D // P
    MGn, MTG = H // MG, MT // 4
    xt_blocks = []
    for i, csz in enumerate(_tile_sizes(C)):
        c0 = i * NTILE
        xt_blocks.append(
            XT.reshape(KT, P, C)[:, :, c0 : c0 + csz].transpose(1, 0, 2).ravel()
        )
    return {
        "xt": np.ascontiguousarray(np.concatenate(xt_blocks)),
        "g": np.ascontiguousarray(G.reshape(1, C)),
        "w1": np.ascontiguousarray(
            w1e.reshape(KT, P, MGn, MG).transpose(2, 1, 0, 3)
        ),
        "b1": np.ascontiguousarray(b1e.reshape(MT, P).T),
        "w2": np.ascontiguousarray(
            w2e.reshape(MTG, 4, P, D).transpose(0, 2, 1, 3)
        ),
        "b2": np.ascontiguousarray(b2e.reshape(DT, P).T),
    }


def _unpack_out(flat, C, D):
    """Blocked per-(n,d) output -> outT [D, C]."""
    DT = D // P
    outT = np.empty((D, C), np.float32)
    off = 0
    for i, csz in enumerate(_tile_sizes(C)):
        c0 = i * NTILE
        for d in range(DT):
            outT[d * P : (d + 1) * P, c0 : c0 + csz] = flat[
                off : off + P * csz
            ].reshape(P, csz)
            off += P * csz
    return outT


def _build_program(C, D, H, mm_dt_name):
    """Build the per-core Bass program (identical on all cores)."""
    import concourse.bass as bass
    import concourse.mybir as mybir
    import concourse.tile as tile
    from concourse import bacc

    f32 = mybir.dt.float32
    mm_dt = getattr(mybir.dt, mm_dt_name)
    act = getattr(mybir.ActivationFunctionType, ACT_FUNC)
    KT = D // P  # 4  k-tiles for matmul1 (contraction over D)
    MT = H // P  # 16 m-tiles (H rows of hT)
    DT = D // P  # 4  d-tiles of the output
    MGn = H // MG  # 4  w1 column blocks
    MTG = MT // 4  # 4  w2 row-block groups
    sizes = _tile_sizes(C)
    NT = len(sizes)

    nc = bacc.Bacc(None, target_bir_lowering=False, debug=False)
    xt_h = nc.dram_tensor("xt", [P * KT * C], mm_dt, kind="ExternalInput")
    g_h = nc.dram_tensor("g", [1, C], f32, kind="ExternalInput")
    w1_h = nc.dram_tensor("w1", [MGn, P, KT, MG], mm_dt, kind="ExternalInput")
    b1_h = nc.dram_tensor("b1", [P, MT], f32, kind="ExternalInput")
    w2_h = nc.dram_tensor("w2", [MTG, P, 4, D], mm_dt, kind="ExternalInput")
    b2_h = nc.dram_tensor("b2", [P, DT], f32, kind="ExternalInput")
    out_h = nc.dram_tensor("out", [P * DT * C], f32, kind="ExternalOutput")

    with tile.TileContext(nc) as tc:
        with (
            tc.tile_pool(name="weights", bufs=1) as wpool,
            tc.tile_pool(name="xio", bufs=2) as xio,
            tc.tile_pool(name="gio", bufs=2) as gio,
            tc.tile_pool(name="oio", bufs=3) as oio,
            tc.tile_pool(name="hbuf", bufs=1) as hbuf,
            tc.tile_pool(name="ps1", bufs=3, space=bass.MemorySpace.PSUM) as ps1,
            # matmul2 keeps DT banks live across its whole m-loop; bufs=1
            # per d-tag (release happens at the DVE evacuation, early in
            # the next n-tile's matmul1 phase). 3 + 4 = 7 of 8 banks.
            tc.tile_pool(name="ps2", bufs=1, space=bass.MemorySpace.PSUM) as ps2,
        ):
            # DMA issue order == consumption order (sync ring is FIFO):
            # xt[n0], w1 blocks, then w2 blocks / biases, then per-n IO.
            # One tile per weight block — Tile deps are per-tile, so a
            # single multi-DMA tile would stall the first matmul on the
            # LAST block's DMA.
            xt_tiles = {}
            xt_tiles[0] = xio.tile([P, KT, sizes[0]], mm_dt, tag="xt", name="xt0")
            nc.sync.dma_start(
                out=xt_tiles[0],
                in_=xt_h.ap()[0 : P * KT * sizes[0]].rearrange(
                    "(p kt c) -> p kt c", p=P, kt=KT
                ),
            )
            w1_t = []
            for mg in range(MGn):
                t = wpool.tile([P, KT, MG], mm_dt, name=f"w1_{mg}")
                nc.sync.dma_start(out=t, in_=w1_h.ap()[mg])
                w1_t.append(t)
            b1_sb = wpool.tile([P, MT], f32)
            nc.sync.dma_start(out=b1_sb, in_=b1_h.ap())
            b2_sb = wpool.tile([P, DT], f32)
            nc.sync.dma_start(out=b2_sb, in_=b2_h.ap())
            w2_t = []
            for mtg in range(MTG):
                t = wpool.tile([P, 4, D], mm_dt, name=f"w2_{mtg}")
                nc.sync.dma_start(out=t, in_=w2_h.ap()[mtg])
                w2_t.append(t)

            xt_off = P * KT * sizes[0]
            out_off = 0
            for n in range(NT):
                csz = sizes[n]
                c0 = n * NTILE
                if n in xt_tiles:
                    xt_t = xt_tiles.pop(n)
                else:
                    xt_t = xio.tile([P, KT, csz], mm_dt, tag="xt", name="xt")
                    nc.sync.dma_start(
                        out=xt_t,
                        in_=xt_h.ap()[xt_off : xt_off + P * KT * csz].rearrange(
                            "(p kt c) -> p kt c", p=P, kt=KT
                        ),
                    )
                    xt_off += P * KT * csz
                g_t = gio.tile([P, csz], f32, tag="g", name="g_t")
                nc.gpsimd.dma_start(
                    out=g_t, in_=g_h.ap()[:, c0 : c0 + csz].partition_broadcast(P)
                )
                hT = hbuf.tile([P, MT, csz], mm_dt, tag="hT", name="hT")
                for m in range(MT):
                    pst = ps1.tile([P, csz], f32, tag="ps1", name="ps1")
                    for kt in range(KT):
                        nc.tensor.matmul(
                            pst,
                            lhsT=w1_t[m // 4][:, kt, (m % 4) * P : (m % 4 + 1) * P],
                            rhs=xt_t[:, kt, :],
                            start=(kt == 0),
                            stop=(kt == KT - 1),
                        )
                    nc.scalar.activation(
                        out=hT[:, m, :],
                        in_=pst,
                        func=act,
                        bias=b1_sb[:, m : m + 1],
                        scale=1.0,
                    )
                # matmul2 with m as the OUTER loop: w2 blocks are consumed
                # in DMA-arrival order, so the first n-tile never stalls on
                # the tail of the weight stream. Needs DT live PSUM banks.
                pso = [
                    ps2.tile([P, csz], f32, tag=f"ps2_{d}", name=f"ps2_{d}")
                    for d in range(DT)
                ]
                for m in range(MT):
                    for d in range(DT):
                        nc.tensor.matmul(
                            pso[d],
                            lhsT=w2_t[m // 4][:, m % 4, d * P : (d + 1) * P],
                            rhs=hT[:, m, :],
                            start=(m == 0),
                            stop=(m == MT - 1),
                        )
                for d in range(DT):
                    ot = oio.tile([P, csz], f32, tag="ot", name="ot")
                    nc.vector.scalar_tensor_tensor(
                        out=ot,
                        in0=pso[d],
                        scalar=b2_sb[:, d : d + 1],
                        in1=g_t,
                        op0=mybir.AluOpType.add,
                        op1=mybir.AluOpType.mult,
                    )
                    nc.sync.dma_start(
                        out=out_h.ap()[out_off : out_off + P * csz].rearrange(
                            "(p c) -> p c", p=P
                        ),
                        in_=ot,
                    )
                    out_off += P * csz

    nc.compile()
    return nc


def _run(nc, in_maps, trace=False):
    from concourse.bass_utils import run_bass_kernel_spmd

    if trace:
        # register the NTFF profiling hook (missing antenv.axon_hooks shim)
        import types

        import antenv

        if not hasattr(antenv, "axon_hooks"):
            mod = types.ModuleType("antenv.axon_hooks")
            _hook = [None]
            mod.set_axon_ntff_profile_hook = lambda h: _hook.__setitem__(0, h)
            mod.get_axon_ntff_profile_hook = lambda: _hook[0]
            sys.modules["antenv.axon_hooks"] = mod
            antenv.axon_hooks = mod
            from trn_agent_boot.trn_boot import _ntff_profile_via_ctypes

            mod.set_axon_ntff_profile_hook(
                _ntff_profile_via_ctypes("/opt/axon/libaxon_pjrt.so")
            )
    return run_bass_kernel_spmd(
        nc, in_maps, core_ids=list(range(N_CORES)), trace=trace
    )


def kernel(x, gate_w, gate_b, w1, b1, w2, b2, _trace=False):
    x = np.ascontiguousarray(np.asarray(x, dtype=np.float32))
    gate_w = np.asarray(gate_w, dtype=np.float32)
    gate_b = np.asarray(gate_b, dtype=np.float32)
    w1 = np.asarray(w1, dtype=np.float32)
    b1 = np.asarray(b1, dtype=np.float32)
    w2 = np.asarray(w2, dtype=np.float32)
    b2 = np.asarray(b2, dtype=np.float32)

    B, S, D = x.shape
    E = gate_w.shape[1]
    H = w1.shape[2]
    assert E == N_CORES
    T = B * S
    x_flat = x.reshape(T, D)

    top_w, top_idx = _route(x_flat, gate_w, gate_b)

    toks, gvals = [], []
    for e in range(E):
        mask = top_idx == e  # [T, K]; at most one True per row
        t_ids = np.nonzero(mask.any(axis=1))[0]
        toks.append(t_ids)
        gvals.append(top_w[mask].astype(np.float32))
    Cmax = max(len(t) for t in toks)
    C = max(((Cmax + P - 1) // P) * P, NTILE)

    in_maps = []
    for e in range(E):
        cnt = len(toks[e])
        XT = np.zeros((D, C), np.float32)
        XT[:, :cnt] = x_flat[toks[e]].T
        G = np.zeros((1, C), np.float32)
        G[0, :cnt] = gvals[e]
        in_maps.append(_pack_inputs(XT, G, w1[e], b1[e], w2[e], b2[e], C, D, H))

    nc = _build_program(C, D, H, MM_DT)
    res = _run(nc, in_maps, trace=_trace)
    global _LAST_RES
    _LAST_RES = res

    out_flat = np.zeros((T, D), np.float32)
    for e in range(E):
        cnt = len(toks[e])
        outT = _unpack_out(res.results[e]["out"], C, D)
        out_flat[toks[e]] += outT[:, :cnt].T

    out = out_flat.reshape(B, S, D)
    if _trace:
        return out, res.exec_time_ns
    return out


# revision 19
# speedup vs baseline: 1.1218x; 1.0141x over previous
"""MoE (top-2 of 8 experts) Trainium2 kernel.

Strategy: expert-parallel across the 8 NeuronCores. The router (a tiny
[T,512]@[512,8] matmul + softmax + top-k, ~0.02% of the layer's FLOPs) runs
on host bit-identically to the reference (jax on CPU). Tokens are gathered
per expert on host, padded to a common capacity C, and each core computes
its expert's full FFN on device:

    outT = (w2.T @ gelu(w1.T @ xT + b1) + b2) * gate

in a transposed layout (features on partitions, tokens on the moving/free
axis) so both matmuls chain on the TensorEngine with no transposes, and the
b1/b2 biases are free per-partition operands. The gate multiply uses a
partition-broadcast gate row. Host scatter-adds the two expert
contributions per token back into the full [B,S,D] output.

Only the selected top-2 experts contribute to the reference output (the
gate is exactly zero elsewhere), so this computes 4x fewer FLOPs than the
dense reference while being numerically equivalent.

All device inputs are packed on host into contiguous ~1MB blocks laid out
in exactly the order the kernel consumes them: HWDGE drains the sync ring
FIFO, so consumption-ordered contiguous blocks give both full DMA
bandwidth and earliest possible compute start.
"""

import os
import sys

sys.path.insert(0, "/opt/trn_rl_repo")

import numpy as np

TOP_K = 2
N_CORES = 8
P = 128  # SBUF partitions

# Matmul dtype: "float32" (exact, 4 cyc/row) or "float32r" (1 cyc/row at
# N>=256, TF32-like internal precision, ~2e-4 rel err end to end).
MM_DT = os.environ.get("MOE_MM_DT", "float32r")
NTILE = 512  # moving-operand (token) tile; max for 4-byte dtypes
MG = 512  # w1 column-block (4 m-tiles per block)
ACT_FUNC = os.environ.get("MOE_ACT_FUNC", "Gelu")  # CoreSim lacks Gelu; Tanh for sim


def _route(x_flat, gate_w, gate_b):
    """Reference router, bit-identical: jax on CPU."""
    import jax
    import jax.numpy as jnp

    with jax.default_device(jax.devices("cpu")[0]):
        logits = jnp.asarray(x_flat) @ jnp.asarray(gate_w) + jnp.asarray(gate_b)
        raw_weights = jax.nn.softmax(logits, axis=-1)
        top_w, top_idx = jax.lax.top_k(raw_weights, TOP_K)
        return np.asarray(top_w), np.asarray(top_idx)


def _tile_sizes(C):
    return [min(NTILE, C - c0) for c0 in range(0, C, NTILE)]


def _pack_inputs(XT, G, w1e, b1e, w2e, b2e, C, D, H):
    """Pack one expert's inputs into the kernel's blocked layouts."""
    KT, MT, DT = D // P, H // P, D // P
    MGn, MTG = H // MG, MT // 4
    xt_blocks = []
    for i, csz in enumerate(_tile_sizes(C)):
        c0 = i * NTILE
        xt_blocks.append(
            XT.reshape(KT, P, C)[:, :, c0 : c0 + csz].transpose(1, 0, 2).ravel()
        )
    return {
        "xt": np.ascontiguousarray(np.concatenate(xt_blocks)),
        "g": np.ascontiguousarray(G.reshape(1, C)),
        "w1": np.ascontiguousarray(
            w1e.reshape(KT, P, MGn, MG).transpose(2, 1, 0, 3)
        ),
        "b1": np.ascontiguousarray(b1e.reshape(MT, P).T),
        "w2": np.ascontiguousarray(
            w2e.reshape(MTG, 4, P, D).transpose(0, 2, 1, 3)
        ),
        "b2": np.ascontiguousarray(b2e.reshape(DT, P).T),
    }


def _unpack_out(flat, C, D):
    """Blocked per-(n,d) output -> outT [D, C]."""
    DT = D // P
    outT = np.empty((D, C), np.float32)
    off = 0
    for i, csz in enumerate(_tile_sizes(C)):
        c0 = i * NTILE
        for d in range(DT):
            outT[d * P : (d + 1) * P, c0 : c0 + csz] = flat[
                off : off + P * csz
            ].reshape(P, csz)
            off += P * csz
    return outT


def _build_program(C, D, H, mm_dt_name):
    """Build the per-core Bass program (identical on all cores)."""
    import concourse.bass as bass
    import concourse.mybir as mybir
    import concourse.tile as tile
    from concourse import bacc
    from concourse.tile_rust import add_dep_helper

    f32 = mybir.dt.float32
    mm_dt = getattr(mybir.dt, mm_dt_name)
    act = getattr(mybir.ActivationFunctionType, ACT_FUNC)
    KT = D // P  # 4  k-tiles for matmul1 (contraction over D)
    MT = H // P  # 16 m-tiles (H rows of hT)
    DT = D // P  # 4  d-tiles of the output
    MGn = H // MG  # 4  w1 column blocks
    MTG = MT // 4  # 4  w2 row-block groups
    sizes = _tile_sizes(C)
    NT = len(sizes)

    nc = bacc.Bacc(None, target_bir_lowering=False, debug=False)
    xt_h = nc.dram_tensor("xt", [P * KT * C], mm_dt, kind="ExternalInput")
    g_h = nc.dram_tensor("g", [1, C], f32, kind="ExternalInput")
    w1_h = nc.dram_tensor("w1", [MGn, P, KT, MG], mm_dt, kind="ExternalInput")
    b1_h = nc.dram_tensor("b1", [P, MT], f32, kind="ExternalInput")
    w2_h = nc.dram_tensor("w2", [MTG, P, 4, D], mm_dt, kind="ExternalInput")
    b2_h = nc.dram_tensor("b2", [P, DT], f32, kind="ExternalInput")
    out_h = nc.dram_tensor("out", [P * DT * C], f32, kind="ExternalOutput")

    with tile.TileContext(nc) as tc:
        with (
            tc.tile_pool(name="weights", bufs=1) as wpool,
            tc.tile_pool(name="xio", bufs=2) as xio,
            tc.tile_pool(name="gio", bufs=2) as gio,
            tc.tile_pool(name="oio", bufs=3) as oio,
            tc.tile_pool(name="hbuf", bufs=1) as hbuf,
            tc.tile_pool(name="ps1", bufs=3, space=bass.MemorySpace.PSUM) as ps1,
            # matmul2 keeps DT banks live across its whole m-loop; bufs=1
            # per d-tag (release happens at the DVE evacuation, early in
            # the next n-tile's matmul1 phase). 3 + 4 = 7 of 8 banks.
            tc.tile_pool(name="ps2", bufs=1, space=bass.MemorySpace.PSUM) as ps2,
        ):
            # DMA issue order == consumption order (sync ring is FIFO):
            # xt[n0], w1 blocks, then w2 blocks / biases, then per-n IO.
            # One tile per weight block — Tile deps are per-tile, so a
            # single multi-DMA tile would stall the first matmul on the
            # LAST block's DMA.
            xt_tiles = {}
            xt_tiles[0] = xio.tile([P, KT, sizes[0]], mm_dt, tag="xt", name="xt0")
            nc.sync.dma_start(
                out=xt_tiles[0],
                in_=xt_h.ap()[0 : P * KT * sizes[0]].rearrange(
                    "(p kt c) -> p kt c", p=P, kt=KT
                ),
            )
            w1_t = []
            for mg in range(MGn):
                t = wpool.tile([P, KT, MG], mm_dt, name=f"w1_{mg}")
                nc.sync.dma_start(out=t, in_=w1_h.ap()[mg])
                w1_t.append(t)
            b1_sb = wpool.tile([P, MT], f32)
            nc.sync.dma_start(out=b1_sb, in_=b1_h.ap())
            b2_sb = wpool.tile([P, DT], f32)
            nc.sync.dma_start(out=b2_sb, in_=b2_h.ap())
            w2_t = []
            w2_dmas = []
            for mtg in range(MTG):
                t = wpool.tile([P, 4, D], mm_dt, name=f"w2_{mtg}")
                w2_dmas.append(nc.sync.dma_start(out=t, in_=w2_h.ap()[mtg]))
                w2_t.append(t)

            xt_off = P * KT * sizes[0]
            out_off = 0
            # DMA-priority gating: everything not needed for the first
            # m-tiles is held back behind early n0 compute, so the ring
            # round-robin doesn't starve the critical xt0+w1 stream.
            gate_act = None  # gelu[n0, m=6]: releases w2 blocks
            prev_first_act = None  # gelu[n-1, m=0]: releases n's xt/g DMAs
            for n in range(NT):
                csz = sizes[n]
                c0 = n * NTILE
                if n in xt_tiles:
                    xt_t = xt_tiles.pop(n)
                else:
                    xt_t = xio.tile([P, KT, csz], mm_dt, tag="xt", name="xt")
                    dma = nc.sync.dma_start(
                        out=xt_t,
                        in_=xt_h.ap()[xt_off : xt_off + P * KT * csz].rearrange(
                            "(p kt c) -> p kt c", p=P, kt=KT
                        ),
                    )
                    if prev_first_act is not None:
                        add_dep_helper(dma.ins, prev_first_act.ins, reason="stagger xt load")
                    xt_off += P * KT * csz
                g_t = gio.tile([P, csz], f32, tag="g", name="g_t")
                dma = nc.gpsimd.dma_start(
                    out=g_t, in_=g_h.ap()[:, c0 : c0 + csz].partition_broadcast(P)
                )
                if prev_first_act is not None:
                    add_dep_helper(dma.ins, prev_first_act.ins, reason="stagger g load")
                hT = hbuf.tile([P, MT, csz], mm_dt, tag="hT", name="hT")
                first_act = None
                for m in range(MT):
                    pst = ps1.tile([P, csz], f32, tag="ps1", name="ps1")
                    for kt in range(KT):
                        nc.tensor.matmul(
                            pst,
                            lhsT=w1_t[m // 4][:, kt, (m % 4) * P : (m % 4 + 1) * P],
                            rhs=xt_t[:, kt, :],
                            start=(kt == 0),
                            stop=(kt == KT - 1),
                        )
                    a = nc.scalar.activation(
                        out=hT[:, m, :],
                        in_=pst,
                        func=act,
                        bias=b1_sb[:, m : m + 1],
                        scale=1.0,
                    )
                    if m == 0:
                        first_act = a
                    if n == 0 and m == 6:
                        gate_act = a
                        for dma in w2_dmas:
                            add_dep_helper(dma.ins, gate_act.ins, reason="stagger w2 load")
                prev_first_act = first_act
                # matmul2 with m as the OUTER loop: w2 blocks are consumed
                # in DMA-arrival order, so the first n-tile never stalls on
                # the tail of the weight stream. Needs DT live PSUM banks.
                pso = [
                    ps2.tile([P, csz], f32, tag=f"ps2_{d}", name=f"ps2_{d}")
                    for d in range(DT)
                ]
                for m in range(MT):
                    for d in range(DT):
                        nc.tensor.matmul(
                            pso[d],
                            lhsT=w2_t[m // 4][:, m % 4, d * P : (d + 1) * P],
                            rhs=hT[:, m, :],
                            start=(m == 0),
                            stop=(m == MT - 1),
                        )
                for d in range(DT):
                    ot = oio.tile([P, csz], f32, tag="ot", name="ot")
                    nc.vector.scalar_tensor_tensor(
                        out=ot,
                        in0=pso[d],
                        scalar=b2_sb[:, d : d + 1],
                        in1=g_t,
                        op0=mybir.AluOpType.add,
                        op1=mybir.AluOpType.mult,
                    )
                    nc.sync.dma_start(
                        out=out_h.ap()[out_off : out_off + P * csz].rearrange(
                            "(p c) -> p c", p=P
                        ),
                        in_=ot,
                    )
                    out_off += P * csz

    nc.compile()
    return nc


def _run(nc, in_maps, trace=False):
    from concourse.bass_utils import run_bass_kernel_spmd

    if trace:
        # register the NTFF profiling hook (missing antenv.axon_hooks shim)
        import types

        import antenv

        if not hasattr(antenv, "axon_hooks"):
            mod = types.ModuleType("antenv.axon_hooks")
            _hook = [None]
            mod.set_axon_ntff_profile_hook = lambda h: _hook.__setitem__(0, h)
            mod.get_axon_ntff_profile_hook = lambda: _hook[0]
            sys.modules["antenv.axon_hooks"] = mod
            antenv.axon_hooks = mod
            from trn_agent_boot.trn_boot import _ntff_profile_via_ctypes

            mod.set_axon_ntff_profile_hook(
                _ntff_profile_via_ctypes("/opt/axon/libaxon_pjrt.so")
            )
    return run_bass_kernel_spmd(
        nc, in_maps, core_ids=list(range(N_CORES)), trace=trace
    )


def kernel(x, gate_w, gate_b, w1, b1, w2, b2, _trace=False):
    x = np.ascontiguousarray(np.asarray(x, dtype=np.float32))
    gate_w = np.asarray(gate_w, dtype=np.float32)
    gate_b = np.asarray(gate_b, dtype=np.float32)
    w1 = np.asarray(w1, dtype=np.float32)
    b1 = np.asarray(b1, dtype=np.float32)
    w2 = np.asarray(w2, dtype=np.float32)
    b2 = np.asarray(b2, dtype=np.float32)

    B, S, D = x.shape
    E = gate_w.shape[1]
    H = w1.shape[2]
    assert E == N_CORES
    T = B * S
    x_flat = x.reshape(T, D)

    top_w, top_idx = _route(x_flat, gate_w, gate_b)

    toks, gvals = [], []
    for e in range(E):
        mask = top_idx == e  # [T, K]; at most one True per row
        t_ids = np.nonzero(mask.any(axis=1))[0]
        toks.append(t_ids)
        gvals.append(top_w[mask].astype(np.float32))
    Cmax = max(len(t) for t in toks)
    C = max(((Cmax + P - 1) // P) * P, NTILE)

    in_maps = []
    for e in range(E):
        cnt = len(toks[e])
        XT = np.zeros((D, C), np.float32)
        XT[:, :cnt] = x_flat[toks[e]].T
        G = np.zeros((1, C), np.float32)
        G[0, :cnt] = gvals[e]
        in_maps.append(_pack_inputs(XT, G, w1[e], b1[e], w2[e], b2[e], C, D, H))

    nc = _build_program(C, D, H, MM_DT)
    res = _run(nc, in_maps, trace=_trace)
    global _LAST_RES
    _LAST_RES = res

    out_flat = np.zeros((T, D), np.float32)
    for e in range(E):
        cnt = len(toks[e])
        outT = _unpack_out(res.results[e]["out"], C, D)
        out_flat[toks[e]] += outT[:, :cnt].T

    out = out_flat.reshape(B, S, D)
    if _trace:
        return out, res.exec_time_ns
    return out


# revision 22
# speedup vs baseline: 1.1299x; 1.0072x over previous
"""MoE (top-2 of 8 experts) Trainium2 kernel.

Strategy: expert-parallel across the 8 NeuronCores. The router (a tiny
[T,512]@[512,8] matmul + softmax + top-k, ~0.02% of the layer's FLOPs) runs
on host bit-identically to the reference (jax on CPU). Tokens are gathered
per expert on host, padded to a common capacity C, and each core computes
its expert's full FFN on device:

    outT = (w2.T @ gelu(w1.T @ xT + b1) + b2) * gate

in a transposed layout (features on partitions, tokens on the moving/free
axis) so both matmuls chain on the TensorEngine with no transposes, and the
b1/b2 biases are free per-partition operands. The gate multiply uses a
partition-broadcast gate row. Host scatter-adds the two expert
contributions per token back into the full [B,S,D] output.

Only the selected top-2 experts contribute to the reference output (the
gate is exactly zero elsewhere), so this computes 4x fewer FLOPs than the
dense reference while being numerically equivalent.

All device inputs are packed on host into contiguous ~1MB blocks laid out
in exactly the order the kernel consumes them: HWDGE drains the sync ring
FIFO, so consumption-ordered contiguous blocks give both full DMA
bandwidth and earliest possible compute start.
"""

import os
import sys

sys.path.insert(0, "/opt/trn_rl_repo")

import numpy as np

TOP_K = 2
N_CORES = 8
P = 128  # SBUF partitions

# Matmul dtype: "float32" (exact, 4 cyc/row) or "float32r" (1 cyc/row at
# N>=256, TF32-like internal precision, ~2e-4 rel err end to end).
MM_DT = os.environ.get("MOE_MM_DT", "float32r")
NTILE = 512  # moving-operand (token) tile; max for 4-byte dtypes
MG = 512  # w1 column-block (4 m-tiles per block)
ACT_FUNC = os.environ.get("MOE_ACT_FUNC", "Gelu")  # CoreSim lacks Gelu; Tanh for sim


def _route(x_flat, gate_w, gate_b):
    """Reference router, bit-identical: jax on CPU."""
    import jax
    import jax.numpy as jnp

    with jax.default_device(jax.devices("cpu")[0]):
        logits = jnp.asarray(x_flat) @ jnp.asarray(gate_w) + jnp.asarray(gate_b)
        raw_weights = jax.nn.softmax(logits, axis=-1)
        top_w, top_idx = jax.lax.top_k(raw_weights, TOP_K)
        return np.asarray(top_w), np.asarray(top_idx)


def _tile_sizes(C):
    return [min(NTILE, C - c0) for c0 in range(0, C, NTILE)]


def _pack_inputs(XT, G, w1e, b1e, w2e, b2e, C, D, H):
    """Pack one expert's inputs into the kernel's blocked layouts."""
    KT, MT, DT = D // P, H // P, D // P
    MGn, MTG = H // MG, MT // 4
    xt_blocks = []
    for i, csz in enumerate(_tile_sizes(C)):
        c0 = i * NTILE
        xt_blocks.append(
            XT.reshape(KT, P, C)[:, :, c0 : c0 + csz].transpose(1, 0, 2).ravel()
        )
    return {
        "xt": np.ascontiguousarray(np.concatenate(xt_blocks)),
        "g": np.ascontiguousarray(G.reshape(1, C)),
        "w1": np.ascontiguousarray(
            w1e.reshape(KT, P, MGn, MG).transpose(2, 1, 0, 3)
        ),
        "b1": np.ascontiguousarray(b1e.reshape(MT, P).T),
        "w2": np.ascontiguousarray(
            w2e.reshape(MTG, 4, P, D).transpose(0, 2, 1, 3)
        ),
        "b2": np.ascontiguousarray(b2e.reshape(DT, P).T),
    }


def _unpack_out(flat, C, D):
    """Blocked per-(n,d) output -> outT [D, C]."""
    DT = D // P
    outT = np.empty((D, C), np.float32)
    off = 0
    for i, csz in enumerate(_tile_sizes(C)):
        c0 = i * NTILE
        for d in range(DT):
            outT[d * P : (d + 1) * P, c0 : c0 + csz] = flat[
                off : off + P * csz
            ].reshape(P, csz)
            off += P * csz
    return outT


def _build_program(C, D, H, mm_dt_name):
    """Build the per-core Bass program (identical on all cores)."""
    import concourse.bass as bass
    import concourse.mybir as mybir
    import concourse.tile as tile
    from concourse import bacc
    from concourse.tile_rust import add_dep_helper

    f32 = mybir.dt.float32
    mm_dt = getattr(mybir.dt, mm_dt_name)
    act = getattr(mybir.ActivationFunctionType, ACT_FUNC)
    KT = D // P  # 4  k-tiles for matmul1 (contraction over D)
    MT = H // P  # 16 m-tiles (H rows of hT)
    DT = D // P  # 4  d-tiles of the output
    MGn = H // MG  # 4  w1 column blocks
    MTG = MT // 4  # 4  w2 row-block groups
    sizes = _tile_sizes(C)
    NT = len(sizes)

    nc = bacc.Bacc(None, target_bir_lowering=False, debug=False)
    xt_h = nc.dram_tensor("xt", [P * KT * C], mm_dt, kind="ExternalInput")
    g_h = nc.dram_tensor("g", [1, C], f32, kind="ExternalInput")
    w1_h = nc.dram_tensor("w1", [MGn, P, KT, MG], mm_dt, kind="ExternalInput")
    b1_h = nc.dram_tensor("b1", [P, MT], f32, kind="ExternalInput")
    w2_h = nc.dram_tensor("w2", [MTG, P, 4, D], mm_dt, kind="ExternalInput")
    b2_h = nc.dram_tensor("b2", [P, DT], f32, kind="ExternalInput")
    out_h = nc.dram_tensor("out", [P * DT * C], f32, kind="ExternalOutput")

    with tile.TileContext(nc) as tc:
        with (
            tc.tile_pool(name="weights", bufs=1) as wpool,
            tc.tile_pool(name="xio", bufs=2) as xio,
            tc.tile_pool(name="gio", bufs=2) as gio,
            tc.tile_pool(name="oio", bufs=3) as oio,
            tc.tile_pool(name="hbuf", bufs=1) as hbuf,
            tc.tile_pool(name="ps1", bufs=3, space=bass.MemorySpace.PSUM) as ps1,
            # matmul2 keeps DT banks live across its whole m-loop; bufs=1
            # per d-tag (release happens at the DVE evacuation, early in
            # the next n-tile's matmul1 phase). 3 + 4 = 7 of 8 banks.
            tc.tile_pool(name="ps2", bufs=1, space=bass.MemorySpace.PSUM) as ps2,
        ):
            # DMA issue order == consumption order (sync ring is FIFO):
            # xt[n0], w1 blocks, then w2 blocks / biases, then per-n IO.
            # One tile per weight block — Tile deps are per-tile, so a
            # single multi-DMA tile would stall the first matmul on the
            # LAST block's DMA.
            xt_tiles = {}
            xt_tiles[0] = xio.tile([P, KT, sizes[0]], mm_dt, tag="xt", name="xt0")
            nc.sync.dma_start(
                out=xt_tiles[0],
                in_=xt_h.ap()[0 : P * KT * sizes[0]].rearrange(
                    "(p kt c) -> p kt c", p=P, kt=KT
                ),
            )
            w1_t = []
            w1_dmas = []
            for mg in range(MGn):
                t = wpool.tile([P, KT, MG], mm_dt, name=f"w1_{mg}")
                w1_dmas.append(nc.sync.dma_start(out=t, in_=w1_h.ap()[mg]))
                w1_t.append(t)
            b1_sb = wpool.tile([P, MT], f32)
            nc.sync.dma_start(out=b1_sb, in_=b1_h.ap())
            b2_sb = wpool.tile([P, DT], f32)
            b2_dma = nc.sync.dma_start(out=b2_sb, in_=b2_h.ap())
            w2_t = []
            w2_dmas = [b2_dma]
            for mtg in range(MTG):
                t = wpool.tile([P, 4, D], mm_dt, name=f"w2_{mtg}")
                w2_dmas.append(nc.sync.dma_start(out=t, in_=w2_h.ap()[mtg]))
                w2_t.append(t)

            xt_off = P * KT * sizes[0]
            out_off = 0
            # DMA-priority gating: everything not needed for the first
            # m-tiles is held back behind early n0 compute, so the ring
            # round-robin doesn't starve the critical xt0+w1 stream.
            gate_act = None  # gelu[n0, m=6]: releases w2 blocks
            prev_first_act = None  # gelu[n-1, m=0]: releases n's xt/g DMAs
            for n in range(NT):
                csz = sizes[n]
                c0 = n * NTILE
                if n in xt_tiles:
                    xt_t = xt_tiles.pop(n)
                else:
                    xt_t = xio.tile([P, KT, csz], mm_dt, tag="xt", name="xt")
                    dma = nc.sync.dma_start(
                        out=xt_t,
                        in_=xt_h.ap()[xt_off : xt_off + P * KT * csz].rearrange(
                            "(p kt c) -> p kt c", p=P, kt=KT
                        ),
                    )
                    if prev_first_act is not None:
                        add_dep_helper(dma.ins, prev_first_act.ins, reason="stagger xt load")
                    xt_off += P * KT * csz
                g_t = gio.tile([P, csz], f32, tag="g", name="g_t")
                dma = nc.gpsimd.dma_start(
                    out=g_t, in_=g_h.ap()[:, c0 : c0 + csz].partition_broadcast(P)
                )
                if prev_first_act is not None:
                    add_dep_helper(dma.ins, prev_first_act.ins, reason="stagger g load")
                hT = hbuf.tile([P, MT, csz], mm_dt, tag="hT", name="hT")
                first_act = None
                for m in range(MT):
                    pst = ps1.tile([P, csz], f32, tag="ps1", name="ps1")
                    for kt in range(KT):
                        mm = nc.tensor.matmul(
                            pst,
                            lhsT=w1_t[m // 4][:, kt, (m % 4) * P : (m % 4 + 1) * P],
                            rhs=xt_t[:, kt, :],
                            start=(kt == 0),
                            stop=(kt == KT - 1),
                        )
                        # just-in-time w1 streaming: block mg+1 is released
                        # by the first matmul that consumes block mg
                        if n == 0 and kt == 0 and m % 4 == 0 and m // 4 + 1 < MGn:
                            add_dep_helper(
                                w1_dmas[m // 4 + 1].ins,
                                mm.ins,
                                reason="stagger w1 load",
                            )
                    a = nc.scalar.activation(
                        out=hT[:, m, :],
                        in_=pst,
                        func=act,
                        bias=b1_sb[:, m : m + 1],
                        scale=1.0,
                    )
                    if m == 0:
                        first_act = a
                    if n == 0 and m == 6:
                        gate_act = a
                        for dma in w2_dmas:
                            add_dep_helper(dma.ins, gate_act.ins, reason="stagger w2 load")
                prev_first_act = first_act
                # matmul2 with m as the OUTER loop: w2 blocks are consumed
                # in DMA-arrival order, so the first n-tile never stalls on
                # the tail of the weight stream. Needs DT live PSUM banks.
                # The LAST tile uses d-outer instead so each d's evacuation
                # and store overlaps the remaining matmuls (shorter tail).
                def evac(pso_d, d):
                    nonlocal out_off
                    ot = oio.tile([P, csz], f32, tag="ot", name="ot")
                    nc.vector.scalar_tensor_tensor(
                        out=ot,
                        in0=pso_d,
                        scalar=b2_sb[:, d : d + 1],
                        in1=g_t,
                        op0=mybir.AluOpType.add,
                        op1=mybir.AluOpType.mult,
                    )
                    nc.sync.dma_start(
                        out=out_h.ap()[out_off : out_off + P * csz].rearrange(
                            "(p c) -> p c", p=P
                        ),
                        in_=ot,
                    )
                    out_off += P * csz

                if n < NT - 1:
                    pso = [
                        ps2.tile([P, csz], f32, tag=f"ps2_{d}", name=f"ps2_{d}")
                        for d in range(DT)
                    ]
                    for m in range(MT):
                        for d in range(DT):
                            nc.tensor.matmul(
                                pso[d],
                                lhsT=w2_t[m // 4][:, m % 4, d * P : (d + 1) * P],
                                rhs=hT[:, m, :],
                                start=(m == 0),
                                stop=(m == MT - 1),
                            )
                    for d in range(DT):
                        evac(pso[d], d)
                else:
                    for d in range(DT):
                        pso_d = ps2.tile(
                            [P, csz], f32, tag=f"ps2_{d}", name=f"ps2_{d}"
                        )
                        for m in range(MT):
                            nc.tensor.matmul(
                                pso_d,
                                lhsT=w2_t[m // 4][:, m % 4, d * P : (d + 1) * P],
                                rhs=hT[:, m, :],
                                start=(m == 0),
                                stop=(m == MT - 1),
                            )
                        evac(pso_d, d)

    nc.compile()
    return nc


def _run(nc, in_maps, trace=False):
    from concourse.bass_utils import run_bass_kernel_spmd

    if trace:
        # register the NTFF profiling hook (missing antenv.axon_hooks shim)
        import types

        import antenv

        if not hasattr(antenv, "axon_hooks"):
            mod = types.ModuleType("antenv.axon_hooks")
            _hook = [None]
            mod.set_axon_ntff_profile_hook = lambda h: _hook.__setitem__(0, h)
            mod.get_axon_ntff_profile_hook = lambda: _hook[0]
            sys.modules["antenv.axon_hooks"] = mod
            antenv.axon_hooks = mod
            from trn_agent_boot.trn_boot import _ntff_profile_via_ctypes

            mod.set_axon_ntff_profile_hook(
                _ntff_profile_via_ctypes("/opt/axon/libaxon_pjrt.so")
            )
    return run_bass_kernel_spmd(
        nc, in_maps, core_ids=list(range(N_CORES)), trace=trace
    )


def kernel(x, gate_w, gate_b, w1, b1, w2, b2, _trace=False):
    x = np.ascontiguousarray(np.asarray(x, dtype=np.float32))
    gate_w = np.asarray(gate_w, dtype=np.float32)
    gate_b = np.asarray(gate_b, dtype=np.float32)
    w1 = np.asarray(w1, dtype=np.float32)
    b1 = np.asarray(b1, dtype=np.float32)
    w2 = np.asarray(w2, dtype=np.float32)
    b2 = np.asarray(b2, dtype=np.float32)

    B, S, D = x.shape
    E = gate_w.shape[1]
    H = w1.shape[2]
    assert E == N_CORES
    T = B * S
    x_flat = x.reshape(T, D)

    top_w, top_idx = _route(x_flat, gate_w, gate_b)

    toks, gvals = [], []
    for e in range(E):
        mask = top_idx == e  # [T, K]; at most one True per row
        t_ids = np.nonzero(mask.any(axis=1))[0]
        toks.append(t_ids)
        gvals.append(top_w[mask].astype(np.float32))
    Cmax = max(len(t) for t in toks)
    C = max(((Cmax + P - 1) // P) * P, NTILE)

    in_maps = []
    for e in range(E):
        cnt = len(toks[e])
        XT = np.zeros((D, C), np.float32)
        XT[:, :cnt] = x_flat[toks[e]].T
        G = np.zeros((1, C), np.float32)
        G[0, :cnt] = gvals[e]
        in_maps.append(_pack_inputs(XT, G, w1[e], b1[e], w2[e], b2[e], C, D, H))

    nc = _build_program(C, D, H, MM_DT)
    res = _run(nc, in_maps, trace=_trace)
    global _LAST_RES
    _LAST_RES = res

    out_flat = np.zeros((T, D), np.float32)
    for e in range(E):
        cnt = len(toks[e])
        outT = _unpack_out(res.results[e]["out"], C, D)
        out_flat[toks[e]] += outT[:, :cnt].T

    out = out_flat.reshape(B, S, D)
    if _trace:
        return out, res.exec_time_ns
    return out


# revision 27
# speedup vs baseline: 1.1779x; 1.0425x over previous
"""MoE (top-2 of 8 experts) Trainium2 kernel.

Strategy: expert-parallel across the 8 NeuronCores. The router (a tiny
[T,512]@[512,8] matmul + softmax + top-k, ~0.02% of the layer's FLOPs) runs
on host bit-identically to the reference (jax on CPU). Tokens are gathered
per expert on host, padded to a common capacity C, and each core computes
its expert's full FFN on device:

    outT = (w2.T @ gelu(w1.T @ xT + b1) + b2) * gate

in a transposed layout (features on partitions, tokens on the moving/free
axis) so both matmuls chain on the TensorEngine with no transposes, and the
b1/b2 biases are free per-partition operands. The gate multiply uses a
partition-broadcast gate row. Host scatter-adds the two expert
contributions per token back into the full [B,S,D] output.

Only the selected top-2 experts contribute to the reference output (the
gate is exactly zero elsewhere), so this computes 4x fewer FLOPs than the
dense reference while being numerically equivalent.

All device inputs are packed on host into contiguous ~1MB blocks laid out
in exactly the order the kernel consumes them: HWDGE drains the sync ring
FIFO, so consumption-ordered contiguous blocks give both full DMA
bandwidth and earliest possible compute start.
"""

import os
import sys

sys.path.insert(0, "/opt/trn_rl_repo")

import numpy as np

TOP_K = 2
N_CORES = 8
P = 128  # SBUF partitions

# Matmul dtype: "float32" (exact, 4 cyc/row) or "float32r" (1 cyc/row at
# N>=256, TF32-like internal precision, ~2e-4 rel err end to end).
MM_DT = os.environ.get("MOE_MM_DT", "float32r")
NTILE = 512  # moving-operand (token) tile; max for 4-byte dtypes
MG = 512  # w1 column-block (4 m-tiles per block)
ACT_FUNC = os.environ.get("MOE_ACT_FUNC", "Gelu")  # CoreSim lacks Gelu; Tanh for sim


def _route(x_flat, gate_w, gate_b):
    """Reference router, bit-identical: jax on CPU."""
    import jax
    import jax.numpy as jnp

    with jax.default_device(jax.devices("cpu")[0]):
        logits = jnp.asarray(x_flat) @ jnp.asarray(gate_w) + jnp.asarray(gate_b)
        raw_weights = jax.nn.softmax(logits, axis=-1)
        top_w, top_idx = jax.lax.top_k(raw_weights, TOP_K)
        return np.asarray(top_w), np.asarray(top_idx)


def _tile_sizes(C):
    return [min(NTILE, C - c0) for c0 in range(0, C, NTILE)]


def _pack_inputs(XT, G, w1e, b1e, w2e, b2e, C, D, H):
    """Pack one expert's inputs into the kernel's blocked layouts."""
    KT, MT, DT = D // P, H // P, D // P
    MGn, MTG = H // MG, MT // 4
    xt_blocks = []
    for i, csz in enumerate(_tile_sizes(C)):
        c0 = i * NTILE
        xt_blocks.append(
            XT.reshape(KT, P, C)[:, :, c0 : c0 + csz].transpose(1, 0, 2).ravel()
        )
    return {
        "xt": np.ascontiguousarray(np.concatenate(xt_blocks)),
        "g": np.ascontiguousarray(G.reshape(1, C)),
        "w1": np.ascontiguousarray(
            w1e.reshape(KT, P, MGn, MG).transpose(2, 1, 0, 3)
        ),
        "b1": np.ascontiguousarray(b1e.reshape(MT, P).T),
        "w2": np.ascontiguousarray(
            w2e.reshape(MTG, 4, P, D).transpose(0, 2, 1, 3)
        ),
        "b2": np.ascontiguousarray(b2e.reshape(DT, P).T),
    }


def _unpack_out(flat, C, D):
    """Blocked per-(n,d) output -> outT [D, C]."""
    DT = D // P
    outT = np.empty((D, C), np.float32)
    off = 0
    for i, csz in enumerate(_tile_sizes(C)):
        c0 = i * NTILE
        for d in range(DT):
            outT[d * P : (d + 1) * P, c0 : c0 + csz] = flat[
                off : off + P * csz
            ].reshape(P, csz)
            off += P * csz
    return outT


def _build_program(C, D, H, mm_dt_name):
    """Build the per-core Bass program (identical on all cores)."""
    import concourse.bass as bass
    import concourse.mybir as mybir
    import concourse.tile as tile
    from concourse import bacc
    from concourse.tile_rust import add_dep_helper

    f32 = mybir.dt.float32
    mm_dt = getattr(mybir.dt, mm_dt_name)
    act = getattr(mybir.ActivationFunctionType, ACT_FUNC)
    KT = D // P  # 4  k-tiles for matmul1 (contraction over D)
    MT = H // P  # 16 m-tiles (H rows of hT)
    DT = D // P  # 4  d-tiles of the output
    MGn = H // MG  # 4  w1 column blocks
    MTG = MT // 4  # 4  w2 row-block groups
    sizes = _tile_sizes(C)
    NT = len(sizes)

    nc = bacc.Bacc(None, target_bir_lowering=False, debug=False)
    xt_h = nc.dram_tensor("xt", [P * KT * C], mm_dt, kind="ExternalInput")
    g_h = nc.dram_tensor("g", [1, C], f32, kind="ExternalInput")
    w1_h = nc.dram_tensor("w1", [MGn, P, KT, MG], mm_dt, kind="ExternalInput")
    b1_h = nc.dram_tensor("b1", [P, MT], f32, kind="ExternalInput")
    w2_h = nc.dram_tensor("w2", [MTG, P, 4, D], mm_dt, kind="ExternalInput")
    b2_h = nc.dram_tensor("b2", [P, DT], f32, kind="ExternalInput")
    out_h = nc.dram_tensor("out", [P * DT * C], f32, kind="ExternalOutput")

    with tile.TileContext(nc) as tc:
        with (
            tc.tile_pool(name="weights", bufs=1) as wpool,
            tc.tile_pool(name="xio", bufs=2) as xio,
            tc.tile_pool(name="gio", bufs=2) as gio,
            tc.tile_pool(name="oio", bufs=3) as oio,
            tc.tile_pool(name="hbuf", bufs=1) as hbuf,
            tc.tile_pool(name="ps1", bufs=3, space=bass.MemorySpace.PSUM) as ps1,
            # matmul2 keeps DT banks live across its whole m-loop; bufs=1
            # per d-tag (release happens at the DVE evacuation, early in
            # the next n-tile's matmul1 phase). 3 + 4 = 7 of 8 banks.
            tc.tile_pool(name="ps2", bufs=1, space=bass.MemorySpace.PSUM) as ps2,
        ):
            # DMA issue order == consumption order (sync ring is FIFO):
            # xt[n0], w1 blocks, then w2 blocks / biases, then per-n IO.
            # One tile per weight block — Tile deps are per-tile, so a
            # single multi-DMA tile would stall the first matmul on the
            # LAST block's DMA.
            # Sync-ring stream order (the sequencer head-of-line-blocks at
            # each gated DMA, serializing everything behind it — by
            # design): tiny g row, xt0, w1_0 and b1 land immediately; the
            # remaining w1 blocks stream just-in-time behind matmul gates;
            # w2/b2/g-broadcast follow once n0's matmul1 is underway.
            xt_tiles = {}
            xt_tiles[0] = xio.tile([P, KT, sizes[0]], mm_dt, tag="xt", name="xt0")
            nc.sync.dma_start(
                out=xt_tiles[0],
                in_=xt_h.ap()[0 : P * KT * sizes[0]].rearrange(
                    "(p kt c) -> p kt c", p=P, kt=KT
                ),
            )
            w1_t = [wpool.tile([P, KT, MG], mm_dt, name=f"w1_{mg}") for mg in range(MGn)]
            w1_dmas = [nc.sync.dma_start(out=w1_t[0], in_=w1_h.ap()[0])]
            b1_sb = wpool.tile([P, MT], f32)
            nc.sync.dma_start(out=b1_sb, in_=b1_h.ap())
            for mg in range(1, MGn):
                w1_dmas.append(nc.sync.dma_start(out=w1_t[mg], in_=w1_h.ap()[mg]))
            b2_sb = wpool.tile([P, DT], f32)
            b2_dma = nc.sync.dma_start(out=b2_sb, in_=b2_h.ap())
            w2_t = []
            w2_dmas = [b2_dma]
            for mtg in range(MTG):
                t = wpool.tile([P, 4, D], mm_dt, name=f"w2_{mtg}")
                w2_dmas.append(nc.sync.dma_start(out=t, in_=w2_h.ap()[mtg]))
                w2_t.append(t)
            # broadcast the gate row across partitions in one HWDGE DMA
            # (reads the 9KB row 128x from HBM; no SWDGE descriptor traffic)
            g_full = gio.tile([P, C], f32, name="g_full")
            nc.sync.dma_start(out=g_full, in_=g_h.ap().partition_broadcast(P))

            xt_off = P * KT * sizes[0]
            out_off = 0
            # DMA-priority gating: everything not needed for the first
            # m-tiles is held back behind early n0 compute, so the ring
            # round-robin doesn't starve the critical xt0+w1 stream.
            gate_act = None  # gelu[n0, m=6]: releases w2 blocks
            prev_first_act = None  # gelu[n-1, m=0]: releases n's xt/g DMAs
            for n in range(NT):
                csz = sizes[n]
                c0 = n * NTILE
                if n in xt_tiles:
                    xt_t = xt_tiles.pop(n)
                else:
                    xt_t = xio.tile([P, KT, csz], mm_dt, tag="xt", name="xt")
                    dma = nc.sync.dma_start(
                        out=xt_t,
                        in_=xt_h.ap()[xt_off : xt_off + P * KT * csz].rearrange(
                            "(p kt c) -> p kt c", p=P, kt=KT
                        ),
                    )
                    if prev_first_act is not None:
                        add_dep_helper(dma.ins, prev_first_act.ins, reason="stagger xt load")
                    xt_off += P * KT * csz
                g_t = g_full[:, c0 : c0 + csz]
                hT = hbuf.tile([P, MT, csz], mm_dt, tag="hT", name="hT")
                first_act = None
                for m in range(MT):
                    pst = ps1.tile([P, csz], f32, tag="ps1", name="ps1")
                    for kt in range(KT):
                        mm = nc.tensor.matmul(
                            pst,
                            lhsT=w1_t[m // 4][:, kt, (m % 4) * P : (m % 4 + 1) * P],
                            rhs=xt_t[:, kt, :],
                            start=(kt == 0),
                            stop=(kt == KT - 1),
                        )
                        # just-in-time w1 streaming: block mg+1 is released
                        # by the first matmul that consumes block mg
                        if n == 0 and kt == 0 and m % 4 == 0 and m // 4 + 1 < MGn:
                            add_dep_helper(
                                w1_dmas[m // 4 + 1].ins,
                                mm.ins,
                                reason="stagger w1 load",
                            )
                    a = nc.scalar.activation(
                        out=hT[:, m, :],
                        in_=pst,
                        func=act,
                        bias=b1_sb[:, m : m + 1],
                        scale=1.0,
                    )
                    if m == 0:
                        first_act = a
                    if n == 0 and m == 6:
                        gate_act = a
                        for dma in w2_dmas:
                            add_dep_helper(dma.ins, gate_act.ins, reason="stagger w2 load")
                prev_first_act = first_act
                # matmul2 with m as the OUTER loop: w2 blocks are consumed
                # in DMA-arrival order, so the first n-tile never stalls on
                # the tail of the weight stream. Needs DT live PSUM banks.
                # The LAST tile uses d-outer instead so each d's evacuation
                # and store overlaps the remaining matmuls (shorter tail).
                def evac(pso_d, d):
                    nonlocal out_off
                    ot = oio.tile([P, csz], f32, tag="ot", name="ot")
                    nc.vector.scalar_tensor_tensor(
                        out=ot,
                        in0=pso_d,
                        scalar=b2_sb[:, d : d + 1],
                        in1=g_t,
                        op0=mybir.AluOpType.add,
                        op1=mybir.AluOpType.mult,
                    )
                    nc.sync.dma_start(
                        out=out_h.ap()[out_off : out_off + P * csz].rearrange(
                            "(p c) -> p c", p=P
                        ),
                        in_=ot,
                    )
                    out_off += P * csz

                if n < NT - 1:
                    pso = [
                        ps2.tile([P, csz], f32, tag=f"ps2_{d}", name=f"ps2_{d}")
                        for d in range(DT)
                    ]
                    for m in range(MT):
                        for d in range(DT):
                            nc.tensor.matmul(
                                pso[d],
                                lhsT=w2_t[m // 4][:, m % 4, d * P : (d + 1) * P],
                                rhs=hT[:, m, :],
                                start=(m == 0),
                                stop=(m == MT - 1),
                            )
                    for d in range(DT):
                        evac(pso[d], d)
                else:
                    for d in range(DT):
                        pso_d = ps2.tile(
                            [P, csz], f32, tag=f"ps2_{d}", name=f"ps2_{d}"
                        )
                        for m in range(MT):
                            nc.tensor.matmul(
                                pso_d,
                                lhsT=w2_t[m // 4][:, m % 4, d * P : (d + 1) * P],
                                rhs=hT[:, m, :],
                                start=(m == 0),
                                stop=(m == MT - 1),
                            )
                        evac(pso_d, d)

    nc.compile()
    return nc


def _run(nc, in_maps, trace=False):
    from concourse.bass_utils import run_bass_kernel_spmd

    if trace:
        # register the NTFF profiling hook (missing antenv.axon_hooks shim)
        import types

        import antenv

        if not hasattr(antenv, "axon_hooks"):
            mod = types.ModuleType("antenv.axon_hooks")
            _hook = [None]
            mod.set_axon_ntff_profile_hook = lambda h: _hook.__setitem__(0, h)
            mod.get_axon_ntff_profile_hook = lambda: _hook[0]
            sys.modules["antenv.axon_hooks"] = mod
            antenv.axon_hooks = mod
            from trn_agent_boot.trn_boot import _ntff_profile_via_ctypes

            mod.set_axon_ntff_profile_hook(
                _ntff_profile_via_ctypes("/opt/axon/libaxon_pjrt.so")
            )
    return run_bass_kernel_spmd(
        nc, in_maps, core_ids=list(range(N_CORES)), trace=trace
    )


def kernel(x, gate_w, gate_b, w1, b1, w2, b2, _trace=False):
    x = np.ascontiguousarray(np.asarray(x, dtype=np.float32))
    gate_w = np.asarray(gate_w, dtype=np.float32)
    gate_b = np.asarray(gate_b, dtype=np.float32)
    w1 = np.asarray(w1, dtype=np.float32)
    b1 = np.asarray(b1, dtype=np.float32)
    w2 = np.asarray(w2, dtype=np.float32)
    b2 = np.asarray(b2, dtype=np.float32)

    B, S, D = x.shape
    E = gate_w.shape[1]
    H = w1.shape[2]
    assert E == N_CORES
    T = B * S
    x_flat = x.reshape(T, D)

    top_w, top_idx = _route(x_flat, gate_w, gate_b)

    toks, gvals = [], []
    for e in range(E):
        mask = top_idx == e  # [T, K]; at most one True per row
        t_ids = np.nonzero(mask.any(axis=1))[0]
        toks.append(t_ids)
        gvals.append(top_w[mask].astype(np.float32))
    Cmax = max(len(t) for t in toks)
    C = max(((Cmax + P - 1) // P) * P, NTILE)

    in_maps = []
    for e in range(E):
        cnt = len(toks[e])
        XT = np.zeros((D, C), np.float32)
        XT[:, :cnt] = x_flat[toks[e]].T
        G = np.zeros((1, C), np.float32)
        G[0, :cnt] = gvals[e]
        in_maps.append(_pack_inputs(XT, G, w1[e], b1[e], w2[e], b2[e], C, D, H))

    nc = _build_program(C, D, H, MM_DT)
    res = _run(nc, in_maps, trace=_trace)
    global _LAST_RES
    _LAST_RES = res

    out_flat = np.zeros((T, D), np.float32)
    for e in range(E):
        cnt = len(toks[e])
        outT = _unpack_out(res.results[e]["out"], C, D)
        out_flat[toks[e]] += outT[:, :cnt].T

    out = out_flat.reshape(B, S, D)
    if _trace:
        return out, res.exec_time_ns
    return out
